# revision 27
# baseline (speedup 1.0000x reference)
"""Trainium2 Bass kernel for the LIF (leaky integrate-and-fire) recurrence.

Reference semantics (fp32, time axis T=64 over state (32, 32768)):
    u_t  = u_{t-1} + 0.5*(x_t - u_{t-1})
    o_t  = (u_t >= 1)
    u_t <- u_t * (1 - o_t)            # spike reset to 0

Device scheme: fixed-point code domain. Host quantizes x to int16 codes
X = rint(x/q) with q = 2/11775.5, so the spike threshold u>=1 (w>=2 in
the doubled-membrane domain) is exactly the integer compare V >= 11776
on the RNE-rounded int16 state (the rounding boundary 11775.5 lands on
the true threshold — no half-LSB bias). Device recurrence on pre-reset
state V (int16):
    V_t = rne(0.5*(V_{t-1} * (V_{t-1} < 11776)) + X_t)    # one DVE op
    o_t = Sign(V_t - 11775.5)  -> int8, +1 iff spike       # ACT, batched
Quantization error vs the f32 reference: 264 spike flips of 67M
(rel err 0.011, gate 2e-2), deterministic.

One fused custom DVE op per step does decode(reset)+integrate; ACT runs
once per 8 steps over the contiguous [128, 8*1024] V-group (amortizes
the 352-cycle ACT fixed cost). Spikes stream out int8 on the SWDGE
queue, input streams in int16 on the sync HWDGE queue. Per-core HBM
traffic: 16.8 MB in + 8.4 MB out (vs 33.6+8.4 for the f32 baseline).

Sharding: pure data parallel; the last axis (32768) is split into 8
chunks of 4096, one per NeuronCore. Per core the (32, 4096) neuron block
is viewed as [128 partitions x 1024 cols].
"""

import sys

import numpy as np

sys.path.insert(0, "/opt/trn_rl_repo")

import concourse.bass as bass  # noqa: E402
import concourse.mybir as mybir  # noqa: E402
from concourse.tile import TileContext  # noqa: E402

T = 64
NB = 32
NN = 32768
NCORES = 8
SH = NN // NCORES  # 4096 neurons (last axis) per core
P = 128
F = (NB * SH) // P  # 1024 columns per partition
GB = 8  # time steps per ACT/output group

F32 = mybir.dt.float32
I16 = mybir.dt.int16
I8 = mybir.dt.int8
Act = mybir.ActivationFunctionType

TH = 11776.0          # integer spike threshold in code domain
Q = 2.0 / 11775.5     # quantization step (w = 2u domain)

# input DMA blocks / ACT+output groups (time steps each). Both start/end
# small so the first compute step isn't stuck behind one large transfer
# and the final ACT+DMA tail after the last DVE step is short. Each block
# and group is its own fully-contiguous DRAM tensor: one InstDMACopy over
# a contiguous region fans across all 16 SDMA engines at near-peak rate,
# unlike the 128 strided per-partition rows a [P, T, F] layout would need.
IN_BLOCKS = [1, 1, 2, 4] + [8] * 7
PRE_SCALAR = ()  # no prefetched blocks: multi-ring input splits measured
PRE_GPS = ()     # slower (ring contention + per-ring single-transfer FIFO)
GROUPS = [8] * 7 + [4, 2, 1, 1]

_LIF_OP = None


def _make_2x_uop():
    """2X_1PORT uop program for the pre-reset LIF step: two packed int16
    elements per 32-bit lane. Element 0 computes in stages 0-3, element 1
    (via the SRC_*_HI input selects) in stages 4-7; element-0's result
    rides delay chain 0 to the write stage. WR0_LO <- result0,
    WR0_HI <- result1. Validated bit-exact on HW (692ns vs 1226ns at 1x
    for [128,1024])."""
    from concourse.dve_uop import (
        AluInp,
        AluOp,
        DelayInp,
        InpSel,
        OutPath,
        OutSel,
        Trigger,
        UopConfig,
    )

    u = UopConfig()
    u.enable_input(InpSel.SRC_0, 1)
    u.enable_input(InpSel.CONST_1, 2)   # th
    u.enable_input(InpSel.CONST_0, 3)   # 0.5
    u.enable_input(InpSel.SRC_1, 4)
    u.enable_input(InpSel.SRC_0_HI, 5)
    u.enable_input(InpSel.SRC_1_HI, 6)
    u.require_inp0 = 1
    u.require_inp1 = 1
    u.trigger = (Trigger.SRC_TENSOR_DONE, Trigger.NONE, Trigger.NONE)
    u.enable_output(OutSel.DELAY_0, OutPath.WR0_LO)   # result0
    u.enable_output(OutSel.ALU_OUT, OutPath.WR0_HI)   # result1

    dp = u.datapath_config
    # dp0: cond0 = (S0 < C1); load all six live values onto delay chains
    dp[0].enable_alu(AluOp.IS_LT, AluInp.PREV_DELAY_0, AluInp.PREV_DELAY_1)
    for c in range(6):  # chain_k <- lane k+1: S0, C1, C0, S1, S0H, S1H
        dp[0].enable_delay_from_src(DelayInp.PREV_DELAY, c)
    # dp1: m0 = cond0 * S0
    dp[1].enable_alu(AluOp.MULTIPLY, AluInp.PREV_ALU_OUT, AluInp.PREV_DELAY_0)
    dp[1].pass_through_delay(1, 2, 3, 4, 5)
    # dp2: h0 = m0 * C0
    dp[2].enable_alu(AluOp.MULTIPLY, AluInp.PREV_ALU_OUT, AluInp.PREV_DELAY_2)
    dp[2].pass_through_delay(1, 2, 3, 4, 5)
    # dp3: result0 = h0 + S1
    dp[3].enable_alu(AluOp.ADD, AluInp.PREV_ALU_OUT, AluInp.PREV_DELAY_3)
    dp[3].pass_through_delay(1, 2, 4, 5)
    # dp4: cond1 = (S0H < C1); save result0 on chain 0
    dp[4].enable_alu(AluOp.IS_LT, AluInp.PREV_DELAY_4, AluInp.PREV_DELAY_1)
    dp[4].enable_delay_from_src(DelayInp.PREV_ALU_OUT, 0)
    dp[4].pass_through_delay(2, 4, 5)
    # dp5: m1 = cond1 * S0H
    dp[5].enable_alu(AluOp.MULTIPLY, AluInp.PREV_ALU_OUT, AluInp.PREV_DELAY_4)
    dp[5].pass_through_delay(0, 2, 5)
    # dp6: h1 = m1 * C0
    dp[6].enable_alu(AluOp.MULTIPLY, AluInp.PREV_ALU_OUT, AluInp.PREV_DELAY_2)
    dp[6].pass_through_delay(0, 5)
    # dp7: result1 = h1 + S1H
    dp[7].enable_alu(AluOp.ADD, AluInp.PREV_ALU_OUT, AluInp.PREV_DELAY_5)
    dp[7].pass_through_delay(0)
    return u


def _get_lif_op():
    """Register (once per process) the fused pre-reset LIF-step DVE op.

        out = ((Src0 < C1) * Src0) * C0 + Src1

    in0 = V_{t-1} (pre-reset state, int16), in1 = X_t (int16 codes),
    C0 = 0.5, C1 = 11776. The compare implements the lazy reset of last
    step's spike (V >= th -> contributes 0); output is this step's
    pre-reset V_t, RNE-rounded/saturated to int16 by the write path.
    4 ALU stages (LT, MULT, MULT, ADD); the table also carries a
    hand-authored 2X_1PORT program (int16 operands run 2 elems/lane/cyc).
    """
    global _LIF_OP
    if _LIF_OP is not None:
        return _LIF_OP
    from concourse import dve_ops
    from concourse.dve_spec import C0, C1, Spec, Src0, Src1, lower
    from concourse.dve_uop import DveOpSpec

    _body = ((Src0 < C1) * Src0) * C0 + Src1

    def _ref(in0, in1, s0, s1, imm2):
        p = in0.astype(np.float32) * (in0 < s1)
        return (p * np.float32(s0) + in1).astype(np.float32)

    spec = Spec(body=_body, reference=_ref)

    class DveOp2x(dve_ops.DveOp):
        """compile() emits the lower()'d 1x program plus the hand-written
        2x variant (lower() cannot generate 2x programs); sha pinning is
        bypassed since both are generated in-process."""

        def compile(self, ver):
            key = (self.name, ver)
            cached = dve_ops._COMPILE_CACHE.get(key)
            if cached is not None:
                return cached
            result = DveOpSpec(
                name=self.name,
                opcode=dve_ops.get_dve_sub_opcode(self.name),
                uops=lower(self.spec, ver=ver),
                uops_2x=[_make_2x_uop()],
                perf_max=1,
                rd1_en=True,
            )
            dve_ops._COMPILE_CACHE[key] = result
            return result

    op = DveOp2x(
        "TENSOR_LEAKY_FIRE_PR",
        spec,
        subdim=False,
        uops_sha={},
    )
    dve_ops.OPS.append(op)
    row = dve_ops._CUSTOM_DVE_ROW_BASE + len(dve_ops.OPS) - 1
    dve_ops._SUB_OPCODE_FOR_NAME[op.name] = row
    dve_ops.CUSTOM_DVE_SPECS[op.name] = op.spec
    _LIF_OP = op
    return op


def build_nc(
    t_steps=T,
    p=P,
    f=F,
    vbufs=3,
    xbufs=4,
    obufs=2,
    in_blocks=None,
    groups=None,
):
    """Build the single-core Bass program (same program runs SPMD on all
    cores). Inputs x0..xN: [p, bsz*f] int16, one contiguous DRAM tensor
    per DMA block; outputs o0..oM: [p, gb*f] int8, one per ACT group."""
    if in_blocks is None:
        in_blocks = IN_BLOCKS
    if groups is None:
        groups = GROUPS
    assert sum(in_blocks) == t_steps
    assert sum(groups) == t_steps

    lif = _get_lif_op()
    nc = bass.Bass()
    xs_dram = [
        nc.dram_tensor(f"x{i}", [p, bsz * f], I16, kind="ExternalInput")
        for i, bsz in enumerate(in_blocks)
    ]
    os_dram = [
        nc.dram_tensor(f"o{i}", [p, gb * f], I8, kind="ExternalOutput")
        for i, gb in enumerate(groups)
    ]

    in_start = {}
    tt = 0
    for i, b in enumerate(in_blocks):
        in_start[tt] = (i, b)
        tt += b

    pre_set = set(PRE_SCALAR) | set(PRE_GPS)
    with TileContext(nc) as tc:
        with (
            tc.tile_pool(name="xp", bufs=xbufs) as xp,
            tc.tile_pool(name="xq", bufs=1) as xpre,
            tc.tile_pool(name="wp", bufs=1) as wp,
            tc.tile_pool(name="vp", bufs=vbufs) as vp,
            tc.tile_pool(name="op", bufs=obufs) as op_,
        ):
            bias = wp.tile([p, 1], F32, tag="bias")
            nc.vector.memset(bias[:], -(TH - 0.5))
            touch = wp.tile([p, 1], F32, tag="touch")
            zero = wp.tile([p, f], I16, tag="zero")
            nc.vector.memset(zero[:], 0.0)
            # the middle blocks are prefetched in full at program start on
            # the scalar HWDGE ring into dedicated buffers: both rings then
            # stream concurrently, and because these issues precede every
            # ACT Sign in the scalar stream, they never stall it (the
            # failure mode of naive sync/scalar alternation). Prefetching
            # the FIRST blocks instead would starve the just-in-time sync
            # stream the chain needs immediately; the middle slice is
            # needed soon enough to matter but late enough not to compete
            # with block 0
            pre_tiles = {}
            for bi, q in [(b, nc.scalar) for b in PRE_SCALAR] + [
                (b, nc.gpsimd) for b in PRE_GPS
            ]:
                bsz = in_blocks[bi]
                pt = xpre.tile([p, bsz * f], I16, tag=f"xpre{bi}")
                q.dma_start(out=pt[:], in_=xs_dram[bi][:, :])
                pre_tiles[bi] = pt
            s_prev = zero[:]
            xt = None
            xt_start = 0
            t = 0
            for gi, gb in enumerate(groups):
                vg = vp.tile([p, GB * f], I16, tag="v")
                for ti in range(gb):
                    if t in in_start:
                        bi, bsz = in_start[t]
                        xt_start = t
                        if bi in pre_set:
                            xt = pre_tiles[bi]
                        else:
                            xt = xp.tile([p, bsz * f], I16, tag="x")
                            # two half-partition transfers back-to-back on
                            # the sync HWDGE ring: the second one streams
                            # while the first's completion ack is still in
                            # flight, hiding the inter-transfer ring gap
                            nc.sync.dma_start(
                                out=xt[: p // 2], in_=xs_dram[bi][: p // 2, :]
                            )
                            nc.sync.dma_start(
                                out=xt[p // 2 :], in_=xs_dram[bi][p // 2 :, :]
                            )
                        # Absorb the DMA-completion wait into a cheap copy so
                        # the fused op below never carries the DMA wait.
                        nc.vector.tensor_copy(touch[:], xt[:, :1])
                    xs = xt[:, (t - xt_start) * f : (t - xt_start + 1) * f]
                    v_new = vg[:, ti * f : (ti + 1) * f]
                    # V_t = rne(0.5*(V_{t-1}*(V_{t-1}<th)) + X_t)
                    nc.vector._custom_dve(
                        lif, out=v_new, in0=s_prev, in1=xs,
                        s0=0.5, s1=TH, imm2=0.0,
                    )
                    s_prev = v_new
                    t += 1
                # one batched Sign over the whole group: +1 iff V >= th
                ot = op_.tile([p, GB * f], I8, tag="o")
                nc.scalar.activation(
                    ot[:, : gb * f], vg[:, : gb * f], Act.Sign, bias=bias[:]
                )
                # outputs ride the scalar ring, issued right after each
                # Sign on the ACT stream: a 1 MB write (~3.3us) hides under
                # the next 7.1us Sign, and gpsimd stays free to stream its
                # input blocks. Final small groups go to the by-then-idle
                # sync ring so no single ring is the last thing draining.
                oq = nc.scalar if t <= t_steps - 4 else nc.sync
                oq.dma_start(out=os_dram[gi][:, :], in_=ot[:, : gb * f])
    return nc


def split_excess_waits(nc, max_waits=1):
    """walrus codegen allows very few sync-wait slots per instruction (the
    STT and pseudo-DMA structs take exactly one). Tile can attach several.
    Hoist the excess onto standalone InstEventSemaphore waits (what raw-bass
    wait_ge emits) placed just before, on the same engine: engines execute
    their stream in order, so semantics are preserved."""
    import bass_rust

    keep_types = ("InstEventSemaphore", "InstAllEngineBarrier")
    # generic raw-ISA instructions carry no sync-wait words
    zero_wait_types = ("InstISA",)
    for fn in nc.m.functions:
        for blk in fn.blocks:
            insts = blk.instructions
            new = []
            changed = False
            for inst in insts:
                si = inst.sync_info
                cap = 0 if type(inst).__name__ in zero_wait_types else max_waits
                if (
                    si is not None
                    and type(inst).__name__ not in keep_types
                    and len(si.on_wait) > cap
                ):
                    waits = list(si.on_wait)
                    extra = waits[: len(waits) - cap]
                    keep = waits[len(waits) - cap :]
                    for k, wt in enumerate(extra):
                        ev = mybir.InstEventSemaphore(
                            name=f"{inst.name}-xw{k}", ins=[], outs=[]
                        )
                        ev.engine = inst.engine
                        ev.sync_info = bass_rust.SyncInfo(
                            on_wait=[wt], on_update=[]
                        )
                        new.append(ev)
                    si.on_wait = keep
                    changed = True
                new.append(inst)
            if changed:
                insts.clear()
                insts.extend(new)
    return nc


_NC = None


def finalize_nc(nc):
    """Post-Tile passes: hoist excess sync waits, enable the 2x perf mode
    on the custom-DVE instructions (byte-36[7:6]; the engine falls back to
    1x at runtime if the mem pattern doesn't qualify), then lower raw-ISA
    subclass instructions to their .instr bytes — raw Bass doesn't run
    this; without it walrus fails with 'ISA wrong length'."""
    split_excess_waits(nc)
    for fn in nc.m.functions:
        for blk in fn.blocks:
            for inst in blk.instructions:
                if type(inst).__name__ == "InstCustomDveAnt":
                    inst.perf_max = 1
    mybir.codegen_inst_isa_subclasses(nc)
    return nc


def _get_nc():
    global _NC
    if _NC is None:
        _NC = finalize_nc(build_nc())
    return _NC


def shard_inputs(ir: np.ndarray) -> list[dict[str, np.ndarray]]:
    ir = np.asarray(ir, dtype=np.float32)
    xq = np.clip(np.rint(ir * np.float32(1.0 / Q)), -32767, 32767).astype(
        np.int16
    )
    maps = []
    for c in range(NCORES):
        xc = xq[:, :, c * SH : (c + 1) * SH].reshape(T, P, F)
        xc = np.ascontiguousarray(xc.transpose(1, 0, 2))  # [P, T, F]
        m = {}
        t = 0
        for i, bsz in enumerate(IN_BLOCKS):
            m[f"x{i}"] = np.ascontiguousarray(
                xc[:, t : t + bsz, :].reshape(P, bsz * F)
            )
            t += bsz
        maps.append(m)
    return maps


def unshard_outputs(results: list[dict[str, np.ndarray]]) -> np.ndarray:
    outs = []
    for c in range(NCORES):
        # [P, T, F] int8, +1 iff spike, reassembled from the group tensors
        oc = np.concatenate(
            [
                results[c][f"o{i}"].reshape(P, gb, F)
                for i, gb in enumerate(GROUPS)
            ],
            axis=1,
        )
        outs.append(oc.transpose(1, 0, 2).reshape(T, NB, SH))
    o = np.concatenate(outs, axis=2)  # (T, NB, NN) int8
    return (o == 1).astype(np.float32)


def run(ir: np.ndarray, trace: bool = False):
    from concourse.bass_utils import run_bass_kernel_spmd

    res = run_bass_kernel_spmd(
        _get_nc(), shard_inputs(ir), list(range(NCORES)), trace=trace
    )
    return unshard_outputs(res.results), res


def kernel(ir: np.ndarray) -> np.ndarray:
    out, _ = run(ir, trace=False)
    return out


# revision 28
# speedup vs baseline: 1.1942x; 1.1942x over previous
"""Trainium2 Bass kernel for the LIF (leaky integrate-and-fire) recurrence.

Reference semantics (fp32, time axis T=64 over state (32, 32768)):
    u_t  = u_{t-1} + 0.5*(x_t - u_{t-1})
    o_t  = (u_t >= 1)
    u_t <- u_t * (1 - o_t)            # spike reset to 0

Device scheme: fixed-point code domain. Host quantizes x to int16 codes
X = rint(x/q) with q = 2/11775.5, so the spike threshold u>=1 (w>=2 in
the doubled-membrane domain) is exactly the integer compare V >= 11776
on the RNE-rounded int16 state (the rounding boundary 11775.5 lands on
the true threshold — no half-LSB bias). Device recurrence on pre-reset
state V (int16):
    V_t = rne(0.5*(V_{t-1} * (V_{t-1} < 11776)) + X_t)    # one DVE op
    o_t = Sign(V_t - 11775.5)  -> int8, +1 iff spike       # ACT, batched
Quantization error vs the f32 reference: 264 spike flips of 67M
(rel err 0.011, gate 2e-2), deterministic.

One fused custom DVE op per step does decode(reset)+integrate; ACT runs
once per 8 steps over the contiguous [128, 8*1024] V-group (amortizes
the 352-cycle ACT fixed cost). Spikes stream out int8 on the SWDGE
queue, input streams in int16 on the sync HWDGE queue. Per-core HBM
traffic: 16.8 MB in + 8.4 MB out (vs 33.6+8.4 for the f32 baseline).

Sharding: pure data parallel; the last axis (32768) is split into 8
chunks of 4096, one per NeuronCore. Per core the (32, 4096) neuron block
is viewed as [128 partitions x 1024 cols].
"""

import sys

import numpy as np

sys.path.insert(0, "/opt/trn_rl_repo")

import concourse.bass as bass  # noqa: E402
import concourse.mybir as mybir  # noqa: E402
from concourse.tile import TileContext  # noqa: E402

T = 64
NB = 32
NN = 32768
NCORES = 8
SH = NN // NCORES  # 4096 neurons (last axis) per core
P = 128
F = (NB * SH) // P  # 1024 columns per partition
GB = 8  # time steps per ACT/output group

F32 = mybir.dt.float32
I16 = mybir.dt.int16
I8 = mybir.dt.int8
Act = mybir.ActivationFunctionType

TH = 11776.0          # integer spike threshold in code domain
Q = 2.0 / 11775.5     # quantization step (w = 2u domain)

# input DMA blocks / ACT+output groups (time steps each). Both start/end
# small so the first compute step isn't stuck behind one large transfer
# and the final ACT+DMA tail after the last DVE step is short. Each block
# and group is its own fully-contiguous DRAM tensor: one InstDMACopy over
# a contiguous region fans across all 16 SDMA engines at near-peak rate,
# unlike the 128 strided per-partition rows a [P, T, F] layout would need.
IN_BLOCKS = [1, 1, 2, 4] + [8] * 6 + [4, 2, 1, 1]
PRE_SCALAR = ()  # no prefetched blocks: multi-ring input splits measured
PRE_GPS = ()     # slower (ring contention + per-ring single-transfer FIFO)
GROUPS = [8] * 7 + [4, 2, 1, 1]

_LIF_OP = None


def _make_2x_uop():
    """2X_1PORT uop program for the pre-reset LIF step: two packed int16
    elements per 32-bit lane. Element 0 computes in stages 0-3, element 1
    (via the SRC_*_HI input selects) in stages 4-7; element-0's result
    rides delay chain 0 to the write stage. WR0_LO <- result0,
    WR0_HI <- result1. Validated bit-exact on HW (692ns vs 1226ns at 1x
    for [128,1024])."""
    from concourse.dve_uop import (
        AluInp,
        AluOp,
        DelayInp,
        InpSel,
        OutPath,
        OutSel,
        Trigger,
        UopConfig,
    )

    u = UopConfig()
    u.enable_input(InpSel.SRC_0, 1)
    u.enable_input(InpSel.CONST_1, 2)   # th
    u.enable_input(InpSel.CONST_0, 3)   # 0.5
    u.enable_input(InpSel.SRC_1, 4)
    u.enable_input(InpSel.SRC_0_HI, 5)
    u.enable_input(InpSel.SRC_1_HI, 6)
    u.require_inp0 = 1
    u.require_inp1 = 1
    u.trigger = (Trigger.SRC_TENSOR_DONE, Trigger.NONE, Trigger.NONE)
    u.enable_output(OutSel.DELAY_0, OutPath.WR0_LO)   # result0
    u.enable_output(OutSel.ALU_OUT, OutPath.WR0_HI)   # result1

    dp = u.datapath_config
    # dp0: cond0 = (S0 < C1); load all six live values onto delay chains
    dp[0].enable_alu(AluOp.IS_LT, AluInp.PREV_DELAY_0, AluInp.PREV_DELAY_1)
    for c in range(6):  # chain_k <- lane k+1: S0, C1, C0, S1, S0H, S1H
        dp[0].enable_delay_from_src(DelayInp.PREV_DELAY, c)
    # dp1: m0 = cond0 * S0
    dp[1].enable_alu(AluOp.MULTIPLY, AluInp.PREV_ALU_OUT, AluInp.PREV_DELAY_0)
    dp[1].pass_through_delay(1, 2, 3, 4, 5)
    # dp2: h0 = m0 * C0
    dp[2].enable_alu(AluOp.MULTIPLY, AluInp.PREV_ALU_OUT, AluInp.PREV_DELAY_2)
    dp[2].pass_through_delay(1, 2, 3, 4, 5)
    # dp3: result0 = h0 + S1
    dp[3].enable_alu(AluOp.ADD, AluInp.PREV_ALU_OUT, AluInp.PREV_DELAY_3)
    dp[3].pass_through_delay(1, 2, 4, 5)
    # dp4: cond1 = (S0H < C1); save result0 on chain 0
    dp[4].enable_alu(AluOp.IS_LT, AluInp.PREV_DELAY_4, AluInp.PREV_DELAY_1)
    dp[4].enable_delay_from_src(DelayInp.PREV_ALU_OUT, 0)
    dp[4].pass_through_delay(2, 4, 5)
    # dp5: m1 = cond1 * S0H
    dp[5].enable_alu(AluOp.MULTIPLY, AluInp.PREV_ALU_OUT, AluInp.PREV_DELAY_4)
    dp[5].pass_through_delay(0, 2, 5)
    # dp6: h1 = m1 * C0
    dp[6].enable_alu(AluOp.MULTIPLY, AluInp.PREV_ALU_OUT, AluInp.PREV_DELAY_2)
    dp[6].pass_through_delay(0, 5)
    # dp7: result1 = h1 + S1H
    dp[7].enable_alu(AluOp.ADD, AluInp.PREV_ALU_OUT, AluInp.PREV_DELAY_5)
    dp[7].pass_through_delay(0)
    return u


def _get_lif_op():
    """Register (once per process) the fused pre-reset LIF-step DVE op.

        out = ((Src0 < C1) * Src0) * C0 + Src1

    in0 = V_{t-1} (pre-reset state, int16), in1 = X_t (int16 codes),
    C0 = 0.5, C1 = 11776. The compare implements the lazy reset of last
    step's spike (V >= th -> contributes 0); output is this step's
    pre-reset V_t, RNE-rounded/saturated to int16 by the write path.
    4 ALU stages (LT, MULT, MULT, ADD); the table also carries a
    hand-authored 2X_1PORT program (int16 operands run 2 elems/lane/cyc).
    """
    global _LIF_OP
    if _LIF_OP is not None:
        return _LIF_OP
    from concourse import dve_ops
    from concourse.dve_spec import C0, C1, Spec, Src0, Src1, lower
    from concourse.dve_uop import DveOpSpec

    _body = ((Src0 < C1) * Src0) * C0 + Src1

    def _ref(in0, in1, s0, s1, imm2):
        p = in0.astype(np.float32) * (in0 < s1)
        return (p * np.float32(s0) + in1).astype(np.float32)

    spec = Spec(body=_body, reference=_ref)

    class DveOp2x(dve_ops.DveOp):
        """compile() emits the lower()'d 1x program plus the hand-written
        2x variant (lower() cannot generate 2x programs); sha pinning is
        bypassed since both are generated in-process."""

        def compile(self, ver):
            key = (self.name, ver)
            cached = dve_ops._COMPILE_CACHE.get(key)
            if cached is not None:
                return cached
            result = DveOpSpec(
                name=self.name,
                opcode=dve_ops.get_dve_sub_opcode(self.name),
                uops=lower(self.spec, ver=ver),
                uops_2x=[_make_2x_uop()],
                perf_max=1,
                rd1_en=True,
            )
            dve_ops._COMPILE_CACHE[key] = result
            return result

    op = DveOp2x(
        "TENSOR_LEAKY_FIRE_PR",
        spec,
        subdim=False,
        uops_sha={},
    )
    dve_ops.OPS.append(op)
    row = dve_ops._CUSTOM_DVE_ROW_BASE + len(dve_ops.OPS) - 1
    dve_ops._SUB_OPCODE_FOR_NAME[op.name] = row
    dve_ops.CUSTOM_DVE_SPECS[op.name] = op.spec
    _LIF_OP = op
    return op


def build_nc(
    t_steps=T,
    p=P,
    f=F,
    vbufs=3,
    xbufs=4,
    obufs=2,
    in_blocks=None,
    groups=None,
):
    """Build the single-core Bass program (same program runs SPMD on all
    cores). Inputs x0..xN: [p, bsz*f] int16, one contiguous DRAM tensor
    per DMA block; outputs o0..oM: [p, gb*f] int8, one per ACT group."""
    if in_blocks is None:
        in_blocks = IN_BLOCKS
    if groups is None:
        groups = GROUPS
    assert sum(in_blocks) == t_steps
    assert sum(groups) == t_steps

    lif = _get_lif_op()
    nc = bass.Bass()
    xs_dram = [
        nc.dram_tensor(f"x{i}", [p, bsz * f], I16, kind="ExternalInput")
        for i, bsz in enumerate(in_blocks)
    ]
    os_dram = [
        nc.dram_tensor(f"o{i}", [p, gb * f], I8, kind="ExternalOutput")
        for i, gb in enumerate(groups)
    ]

    in_start = {}
    tt = 0
    for i, b in enumerate(in_blocks):
        in_start[tt] = (i, b)
        tt += b

    pre_set = set(PRE_SCALAR) | set(PRE_GPS)
    with TileContext(nc) as tc:
        with (
            tc.tile_pool(name="xp", bufs=xbufs) as xp,
            tc.tile_pool(name="xq", bufs=1) as xpre,
            tc.tile_pool(name="wp", bufs=1) as wp,
            tc.tile_pool(name="vp", bufs=vbufs) as vp,
            tc.tile_pool(name="op", bufs=obufs) as op_,
        ):
            bias = wp.tile([p, 1], F32, tag="bias")
            nc.vector.memset(bias[:], -(TH - 0.5))
            touch = wp.tile([p, 1], F32, tag="touch")
            zero = wp.tile([p, f], I16, tag="zero")
            nc.vector.memset(zero[:], 0.0)
            # the middle blocks are prefetched in full at program start on
            # the scalar HWDGE ring into dedicated buffers: both rings then
            # stream concurrently, and because these issues precede every
            # ACT Sign in the scalar stream, they never stall it (the
            # failure mode of naive sync/scalar alternation). Prefetching
            # the FIRST blocks instead would starve the just-in-time sync
            # stream the chain needs immediately; the middle slice is
            # needed soon enough to matter but late enough not to compete
            # with block 0
            pre_tiles = {}
            for bi, q in [(b, nc.scalar) for b in PRE_SCALAR] + [
                (b, nc.gpsimd) for b in PRE_GPS
            ]:
                bsz = in_blocks[bi]
                pt = xpre.tile([p, bsz * f], I16, tag=f"xpre{bi}")
                q.dma_start(out=pt[:], in_=xs_dram[bi][:, :])
                pre_tiles[bi] = pt
            s_prev = zero[:]
            xt = None
            xt_start = 0
            t = 0
            for gi, gb in enumerate(groups):
                vg = vp.tile([p, GB * f], I16, tag="v")
                for ti in range(gb):
                    if t in in_start:
                        bi, bsz = in_start[t]
                        xt_start = t
                        if bi in pre_set:
                            xt = pre_tiles[bi]
                        else:
                            xt = xp.tile([p, bsz * f], I16, tag="x")
                            # leading blocks stream just-in-time on the sync
                            # HWDGE ring (pairing half-partition transfers
                            # measured slower: the ring serializes them)
                            nc.sync.dma_start(
                                out=xt[:], in_=xs_dram[bi][:, :]
                            )
                        # Absorb the DMA-completion wait into a cheap copy so
                        # the fused op below never carries the DMA wait.
                        nc.vector.tensor_copy(touch[:], xt[:, :1])
                    xs = xt[:, (t - xt_start) * f : (t - xt_start + 1) * f]
                    v_new = vg[:, ti * f : (ti + 1) * f]
                    # V_t = rne(0.5*(V_{t-1}*(V_{t-1}<th)) + X_t)
                    nc.vector._custom_dve(
                        lif, out=v_new, in0=s_prev, in1=xs,
                        s0=0.5, s1=TH, imm2=0.0,
                    )
                    s_prev = v_new
                    t += 1
                # one batched Sign over the whole group: +1 iff V >= th
                ot = op_.tile([p, GB * f], I8, tag="o")
                nc.scalar.activation(
                    ot[:, : gb * f], vg[:, : gb * f], Act.Sign, bias=bias[:]
                )
                # outputs ride the scalar ring, issued right after each
                # Sign on the ACT stream: a 1 MB write (~3.3us) hides under
                # the next 7.1us Sign, and gpsimd stays free to stream its
                # input blocks. Final small groups go to the by-then-idle
                # sync ring so no single ring is the last thing draining.
                oq = nc.scalar if t <= t_steps - 4 else nc.sync
                oq.dma_start(out=os_dram[gi][:, :], in_=ot[:, : gb * f])
    return nc


def split_excess_waits(nc, max_waits=1):
    """walrus codegen allows very few sync-wait slots per instruction (the
    STT and pseudo-DMA structs take exactly one). Tile can attach several.
    Hoist the excess onto standalone InstEventSemaphore waits (what raw-bass
    wait_ge emits) placed just before, on the same engine: engines execute
    their stream in order, so semantics are preserved."""
    import bass_rust

    keep_types = ("InstEventSemaphore", "InstAllEngineBarrier")
    # generic raw-ISA instructions carry no sync-wait words
    zero_wait_types = ("InstISA",)
    for fn in nc.m.functions:
        for blk in fn.blocks:
            insts = blk.instructions
            new = []
            changed = False
            for inst in insts:
                si = inst.sync_info
                cap = 0 if type(inst).__name__ in zero_wait_types else max_waits
                if (
                    si is not None
                    and type(inst).__name__ not in keep_types
                    and len(si.on_wait) > cap
                ):
                    waits = list(si.on_wait)
                    extra = waits[: len(waits) - cap]
                    keep = waits[len(waits) - cap :]
                    for k, wt in enumerate(extra):
                        ev = mybir.InstEventSemaphore(
                            name=f"{inst.name}-xw{k}", ins=[], outs=[]
                        )
                        ev.engine = inst.engine
                        ev.sync_info = bass_rust.SyncInfo(
                            on_wait=[wt], on_update=[]
                        )
                        new.append(ev)
                    si.on_wait = keep
                    changed = True
                new.append(inst)
            if changed:
                insts.clear()
                insts.extend(new)
    return nc


_NC = None


def finalize_nc(nc):
    """Post-Tile passes: hoist excess sync waits, enable the 2x perf mode
    on the custom-DVE instructions (byte-36[7:6]; the engine falls back to
    1x at runtime if the mem pattern doesn't qualify), then lower raw-ISA
    subclass instructions to their .instr bytes — raw Bass doesn't run
    this; without it walrus fails with 'ISA wrong length'."""
    split_excess_waits(nc)
    for fn in nc.m.functions:
        for blk in fn.blocks:
            for inst in blk.instructions:
                if type(inst).__name__ == "InstCustomDveAnt":
                    inst.perf_max = 1
    mybir.codegen_inst_isa_subclasses(nc)
    return nc


def _get_nc():
    global _NC
    if _NC is None:
        _NC = finalize_nc(build_nc())
    return _NC


def shard_inputs(ir: np.ndarray) -> list[dict[str, np.ndarray]]:
    ir = np.asarray(ir, dtype=np.float32)
    xq = np.clip(np.rint(ir * np.float32(1.0 / Q)), -32767, 32767).astype(
        np.int16
    )
    maps = []
    for c in range(NCORES):
        xc = xq[:, :, c * SH : (c + 1) * SH].reshape(T, P, F)
        xc = np.ascontiguousarray(xc.transpose(1, 0, 2))  # [P, T, F]
        m = {}
        t = 0
        for i, bsz in enumerate(IN_BLOCKS):
            m[f"x{i}"] = np.ascontiguousarray(
                xc[:, t : t + bsz, :].reshape(P, bsz * F)
            )
            t += bsz
        maps.append(m)
    return maps


def unshard_outputs(results: list[dict[str, np.ndarray]]) -> np.ndarray:
    outs = []
    for c in range(NCORES):
        # [P, T, F] int8, +1 iff spike, reassembled from the group tensors
        oc = np.concatenate(
            [
                results[c][f"o{i}"].reshape(P, gb, F)
                for i, gb in enumerate(GROUPS)
            ],
            axis=1,
        )
        outs.append(oc.transpose(1, 0, 2).reshape(T, NB, SH))
    o = np.concatenate(outs, axis=2)  # (T, NB, NN) int8
    return (o == 1).astype(np.float32)


def run(ir: np.ndarray, trace: bool = False):
    from concourse.bass_utils import run_bass_kernel_spmd

    res = run_bass_kernel_spmd(
        _get_nc(), shard_inputs(ir), list(range(NCORES)), trace=trace
    )
    return unshard_outputs(res.results), res


def kernel(ir: np.ndarray) -> np.ndarray:
    out, _ = run(ir, trace=False)
    return out
